# revision 15
# baseline (speedup 1.0000x reference)
"""Trainium2 Bass kernel for AngularFeaturePropagation (retrieval_knn).

Computation per batch element b (one NeuronCore per b, B=8 = n_cores):
  idx[n]  = argmin_m sqrt((lt[n]-ht[m])^2 + (lp[n]-hp[m])^2)      n<8192, m<2048
  interp  = high_feats[:, idx]                                     [128, 8192]
  cat     = [low_feats; interp]                                    [192, 8192]
  y0 = W0 @ cat  -> BN(global batch stats, over all cores) -> ReLU
  y1 = W1 @ h0   -> BN -> ReLU -> out                              [128, 8192]

Device algorithm (v3):
  - scores s[n,m] = 2*lt*ht + 2*lp*hp - (ht^2+hp^2)  (= -dist^2 + const(n))
    via a K=15 bf16 PE matmul (1 cycle/row): every fp32 factor is split
    exactly into hi+mid+lo bf16 terms on the HOST (24 mantissa bits, zero
    split residual), so each bf16 product is exact and only the fp32 PSUM
    accumulation rounds -> near-fp32 scores at bf16 matmul speed.
  - argmax via prefix-max scan + sign-count:
      pm = chained tensor_tensor_scan(max) over the 2048 candidates
      (DVE, the only PSUM consumer -> PSUM freed early);
      idx = #{t : pm[t] < pm[last]} = sum of Sign(pm[last] - pm[t])
      accumulated by the Activation engine's accum_out. Exact fp32
      argmax-first semantics; no rescore stage (argmax of the
      expanded-form scores flips ~2/65536 queries vs the reference).
  - feature gather via GPSIMD ap_gather (channel-major fp32), per
    2-batch chunk, overlapped with the remaining score tiles.
  - MLP in fp16 (weights + low_feats pre-cast on host; interp chunks
    cast by ScalarE; h0 written fp16 by the fused BN0 apply). Layer-0
    BN batch sums (sum y, sum y^2) ride on ScalarE accum_out during the
    PSUM->SBUF copy and a Square pass; layer-1 uses DVE bn_stats in the
    tail. BN: AllReduce of per-device sums; biases dropped (BN
    shift-invariant); affine+ReLU fused into one ScalarE activation.
"""

import sys

if '/opt/trn_rl_repo' not in sys.path:
    sys.path.insert(0, '/opt/trn_rl_repo')

import numpy as np

import concourse.bass as bass
import concourse.bacc as bacc
import concourse.tile as tile
import concourse.mybir as mybir
from concourse import bass_utils, library_config

F32 = mybir.dt.float32
BF16 = mybir.dt.bfloat16
F16 = mybir.dt.float16
U16 = mybir.dt.uint16
I16 = mybir.dt.int16
AF = mybir.ActivationFunctionType
OP = mybir.AluOpType
AX = mybir.AxisListType

B, N, M, C1, C2 = 8, 8192, 2048, 64, 128
NT = N // 128          # 64 query tiles
NCH = N // 512         # 16 MLP chunks
K15 = 15               # bf16 split rows
EPS = 1e-5


def build(num_devices=8):
    nc = bacc.Bacc("TRN2", target_bir_lowering=False, debug=False, num_devices=num_devices)

    # ---- per-core DRAM I/O ----
    d_qf = nc.dram_tensor("qf15", [K15, N], BF16, kind="ExternalInput")
    d_cf = nc.dram_tensor("cf15", [K15, M], BF16, kind="ExternalInput")
    d_lf = nc.dram_tensor("lf16", [C1, N], F16, kind="ExternalInput")
    d_hf = nc.dram_tensor("hf", [C2, M], F32, kind="ExternalInput")
    d_w0lot = nc.dram_tensor("w0lot", [C1, 128], F16, kind="ExternalInput")
    d_w0hit = nc.dram_tensor("w0hit", [C2, 128], F16, kind="ExternalInput")
    d_w1t = nc.dram_tensor("w1t", [128, 128], F16, kind="ExternalInput")
    d_g0 = nc.dram_tensor("g0", [128, 1], F32, kind="ExternalInput")
    d_be0 = nc.dram_tensor("be0", [128, 1], F32, kind="ExternalInput")
    d_g1 = nc.dram_tensor("g1", [128, 1], F32, kind="ExternalInput")
    d_be1 = nc.dram_tensor("be1", [128, 1], F32, kind="ExternalInput")
    d_out = nc.dram_tensor("out", [128, N], F32, kind="ExternalOutput")

    with tile.TileContext(nc) as tc:
        with (
            tc.tile_pool(name="persist", bufs=1) as persist,
            tc.tile_pool(name="dram", bufs=1, space="DRAM") as dram,
            tc.tile_pool(name="small", bufs=2) as small,
            tc.tile_pool(name="lfq", bufs=2) as lfp,
            tc.tile_pool(name="pm", bufs=3) as pmp,
            tc.tile_pool(name="junk", bufs=3) as junkp,
            tc.tile_pool(name="ojunk", bufs=2) as ojunkp,
            tc.tile_pool(name="i16", bufs=2) as i16p,
            tc.tile_pool(name="osb", bufs=2) as osbp,
        ):
            # ---------------- stage 0: loads ----------------
            nc.gpsimd.load_library(library_config.ap_gather)
            qfeat = persist.tile([K15, N], BF16)
            nc.sync.dma_start(qfeat[:], d_qf.ap())
            candfeat = persist.tile([K15, M], BF16)
            nc.sync.dma_start(candfeat[:], d_cf.ap())
            hf_sb = persist.tile([C2, M], F32)
            nc.sync.dma_start(hf_sb[:], d_hf.ap())
            w0lot = persist.tile([C1, 128], F16)
            nc.sync.dma_start(w0lot[:], d_w0lot.ap())
            w0hit = persist.tile([C2, 128], F16)
            nc.sync.dma_start(w0hit[:], d_w0hit.ap())
            w1t = persist.tile([128, 128], F16)
            nc.sync.dma_start(w1t[:], d_w1t.ap())
            g0 = persist.tile([128, 1], F32)
            nc.sync.dma_start(g0[:], d_g0.ap())
            be0 = persist.tile([128, 1], F32)
            nc.sync.dma_start(be0[:], d_be0.ap())
            g1 = persist.tile([128, 1], F32)
            nc.sync.dma_start(g1[:], d_g1.ap())
            be1 = persist.tile([128, 1], F32)
            nc.sync.dma_start(be1[:], d_be1.ap())

            # scan data1 operand (ignored under op1=bypass, but must exist)
            zeros16 = persist.tile([128, 1024], F16)
            nc.vector.memset(zeros16[:], 0.0)

            jall = persist.tile([128, NT], F32)
            idx_u = persist.tile([128, NT], U16)
            d_fi = dram.tile([N], U16)
            fiw = persist.tile([128, N // 16], U16)

            sy0 = persist.tile([128, NCH], F32)
            sy0sq = persist.tile([128, NCH], F32)
            st1 = persist.tile([128, NCH * 6], F32)

            interp = persist.tile([C2, N], F32)
            interp3 = interp[:].rearrange("p (m d) -> p m d", d=1)
            hf3 = hf_sb[:].rearrange("p (m d) -> p m d", d=1)
            y0 = persist.tile([128, N], F16)
            h0 = persist.tile([128, N], F16)
            y1 = persist.tile([128, N], F16)

            # ---------------- phase 1: scores + argmax + L0 ----------------
            with (
                tc.tile_pool(name="spsum", bufs=3, space="PSUM") as spsum,
                tc.tile_pool(name="mpsum", bufs=2, space="PSUM") as mpsum,
            ):
                def l0_chunk(c):
                    # layer-0 matmuls + copies + BN sums for 512-token chunk c
                    q = c // 4
                    if c % 4 == 0:
                        lfq = lfp.tile([C1, 2048], F16, tag="lfq")
                        nc.sync.dma_start(lfq[:], d_lf.ap()[:, 2048 * q:2048 * (q + 1)])
                        l0_chunk.lfq = lfq
                    lfch = l0_chunk.lfq[:, 512 * (c % 4):512 * (c % 4 + 1)]
                    i16 = i16p.tile([128, 512], F16, tag="i16")
                    nc.gpsimd.tensor_copy(i16[:], interp[:, 512 * c:512 * (c + 1)])
                    mps = mpsum.tile([128, 512], F32)
                    nc.tensor.matmul(mps[:], w0lot[:], lfch, start=True, stop=False)
                    nc.tensor.matmul(mps[:], w0hit[:], i16[:], start=False, stop=True)
                    nc.scalar.activation(y0[:, 512 * c:512 * (c + 1)], mps[:], AF.Copy,
                                         accum_out=sy0[:, c:c + 1])
                    oj = ojunkp.tile([128, 512], F16, tag="oj")
                    nc.scalar.activation(oj[:], mps[:], AF.Square,
                                         accum_out=sy0sq[:, c:c + 1])

                for t in range(NT):
                    qT = qfeat[:, 128 * t:128 * (t + 1)]
                    psA = spsum.tile([128, 1024], F32, tag="ps")
                    nc.tensor.matmul(psA[:, 0:512], qT, candfeat[:, 0:512],
                                     start=True, stop=True)
                    nc.tensor.matmul(psA[:, 512:1024], qT, candfeat[:, 512:1024],
                                     start=True, stop=True)
                    psB = spsum.tile([128, 1024], F32, tag="ps")
                    nc.tensor.matmul(psB[:, 0:512], qT, candfeat[:, 1024:1536],
                                     start=True, stop=True)
                    nc.tensor.matmul(psB[:, 512:1024], qT, candfeat[:, 1536:2048],
                                     start=True, stop=True)
                    pm = pmp.tile([128, M], F32, tag="pm")
                    nc.vector.tensor_tensor_scan(pm[:, 0:1024], psA[:], zeros16[:],
                                                 initial=-3.0e38, op0=OP.max, op1=OP.bypass)
                    nc.vector.tensor_tensor_scan(pm[:, 1024:2048], psB[:], zeros16[:],
                                                 initial=pm[:, 1023:1024],
                                                 op0=OP.max, op1=OP.bypass)
                    junk = junkp.tile([128, M], F16, tag="junk")
                    nc.scalar.activation(junk[:], pm[:], AF.Sign,
                                         bias=pm[:, 2047:2048], scale=-1.0,
                                         accum_out=jall[:, t:t + 1])

                    if t % 8 == 3 and t // 8 >= 1:
                        # first deferred layer-0 chunk of the previous batch
                        l0_chunk(2 * (t // 8 - 1))
                    if t % 8 == 7:
                        # batch j of 8 tiles: indices final -> stage + gather;
                        # the previous batch's remaining layer-0 chunk runs now
                        # (one-batch deferral keeps Act's in-order queue from
                        # head-blocking on gather latency).
                        j = t // 8
                        nc.vector.tensor_copy(idx_u[:, 8 * j:8 * (j + 1)],
                                              jall[:, 8 * j:8 * (j + 1)])
                        nc.sync.dma_start(
                            d_fi[1024 * j:1024 * (j + 1)].rearrange("(t p) -> p t", p=128),
                            idx_u[:, 8 * j:8 * (j + 1)])
                        for g in range(8):
                            nc.sync.dma_start(
                                fiw[16 * g:16 * (g + 1), 64 * j:64 * (j + 1)],
                                d_fi[1024 * j:1024 * (j + 1)].rearrange("(s p) -> p s", p=16))
                        nc.gpsimd.ap_gather(
                            interp3[:, 1024 * j:1024 * (j + 1), :], hf3,
                            fiw[:, 64 * j:64 * (j + 1)].bitcast(I16),
                            channels=128, num_elems=M, d=1, num_idxs=1024,
                        )
                        if j >= 1:
                            l0_chunk(2 * (j - 1) + 1)

                # last batch's layer-0 chunks
                l0_chunk(NCH - 2)
                l0_chunk(NCH - 1)

                # ---------------- tail: BN0 -> L1 -> BN1 -> out ----------------
                d_ccin = dram.tile([128, 2], F32)
                d_ccout = dram.tile([128, 2], F32)
                d_ccin1 = dram.tile([128, 2], F32)
                d_ccout1 = dram.tile([128, 2], F32)

                def bn_coeffs(cc_sb, d_in, d_out, gam, bet, tag):
                    # cc_sb [128,2] = per-device (sum, sumsq); AllReduce-add
                    # then mean/E2 -> scale sc, shift sh for fused affine+ReLU.
                    nc.sync.dma_start(d_in[:], cc_sb[:])
                    if tag == "b0":
                        # pull the Sqrt act-table load under the collective wait
                        sqj = small.tile([128, 1], F32, tag="sqj")
                        nc.vector.memset(sqj[:], 1.0)
                        nc.scalar.activation(sqj[:], sqj[:], AF.Sqrt)
                    if num_devices > 1:
                        nc.gpsimd.collective_compute(
                            "AllReduce", OP.add,
                            replica_groups=[list(range(num_devices))],
                            ins=[d_in[:].opt()], outs=[d_out[:].opt()],
                        )
                    else:
                        nc.sync.dma_start(d_out[:], d_in[:])
                    ccr = small.tile([128, 2], F32, tag=tag + "ccr")
                    nc.sync.dma_start(ccr[:], d_out[:])
                    mu = small.tile([128, 1], F32, tag=tag + "mu")
                    nc.vector.tensor_scalar_mul(mu[:], ccr[:, 0:1], 1.0 / (num_devices * N))
                    e2g = small.tile([128, 1], F32, tag=tag + "e2g")
                    nc.vector.tensor_scalar_mul(e2g[:], ccr[:, 1:2], 1.0 / (num_devices * N))
                    musq = small.tile([128, 1], F32, tag=tag + "musq")
                    nc.vector.tensor_mul(musq[:], mu[:], mu[:])
                    var = small.tile([128, 1], F32, tag=tag + "var")
                    nc.vector.tensor_sub(var[:], e2g[:], musq[:])
                    vpe = small.tile([128, 1], F32, tag=tag + "vpe")
                    nc.vector.tensor_scalar_add(vpe[:], var[:], EPS)
                    sd = small.tile([128, 1], F32, tag=tag + "sd")
                    nc.scalar.activation(sd[:], vpe[:], AF.Sqrt)
                    rs = small.tile([128, 1], F32, tag=tag + "rs")
                    nc.vector.reciprocal(rs[:], sd[:])
                    sc = small.tile([128, 1], F32, tag=tag + "sc")
                    nc.vector.tensor_mul(sc[:], gam[:], rs[:])
                    msc = small.tile([128, 1], F32, tag=tag + "msc")
                    nc.vector.tensor_mul(msc[:], mu[:], sc[:])
                    sh = small.tile([128, 1], F32, tag=tag + "sh")
                    nc.vector.tensor_sub(sh[:], bet[:], msc[:])
                    return sc, sh

                # layer-0 stats: sums accumulated on Act during phase 1
                cc0 = small.tile([128, 2], F32, tag="cc0")
                nc.vector.tensor_reduce(cc0[:, 0:1], sy0[:], axis=AX.X, op=OP.add)
                nc.vector.tensor_reduce(cc0[:, 1:2], sy0sq[:], axis=AX.X, op=OP.add)
                sc0, sh0 = bn_coeffs(cc0, d_ccin, d_ccout, g0, be0, "b0")

                # BN0 apply (chunked, h0 in fp16); L1 matmuls + y1 + bn_stats follow
                for qq in range(4):
                    s2 = slice(2048 * qq, 2048 * (qq + 1))
                    nc.scalar.activation(h0[:, s2], y0[:, s2], AF.Relu,
                                         bias=sh0[:], scale=sc0[:])
                    for c in range(4 * qq, 4 * qq + 4):
                        mps = mpsum.tile([128, 512], F32)
                        nc.tensor.matmul(mps[:], w1t[:], h0[:, 512 * c:512 * (c + 1)],
                                         start=True, stop=True)
                        nc.vector.bn_stats(st1[:, 6 * c:6 * (c + 1)], mps[:])
                        nc.scalar.activation(y1[:, 512 * c:512 * (c + 1)], mps[:], AF.Copy)

                # layer-1 stats from bn_stats/bn_aggr -> (sum, sumsq)
                ag1 = small.tile([128, 2], F32, tag="ag1")
                nc.vector.bn_aggr(ag1[:], st1[:])
                msq1 = small.tile([128, 1], F32, tag="msq1")
                nc.vector.tensor_mul(msq1[:], ag1[:, 0:1], ag1[:, 0:1])
                cc1 = small.tile([128, 2], F32, tag="cc1")
                nc.vector.tensor_scalar_mul(cc1[:, 0:1], ag1[:, 0:1], float(N))
                e2s = small.tile([128, 1], F32, tag="e2s")
                nc.vector.tensor_add(e2s[:], ag1[:, 1:2], msq1[:])
                nc.vector.tensor_scalar_mul(cc1[:, 1:2], e2s[:], float(N))
                sc1, sh1 = bn_coeffs(cc1, d_ccin1, d_ccout1, g1, be1, "b1")

                # BN1 apply + store (1024-wide so the serial store DMA starts early)
                for qq in range(8):
                    s2 = slice(1024 * qq, 1024 * (qq + 1))
                    o_sb = osbp.tile([128, 1024], F32, tag="osb")
                    nc.scalar.activation(o_sb[:], y1[:, s2], AF.Relu,
                                         bias=sh1[:], scale=sc1[:])
                    nc.sync.dma_start(d_out.ap()[:, s2], o_sb[:])

    nc.compile()
    return nc


_NC_CACHE = None


def _get_nc():
    global _NC_CACHE
    if _NC_CACHE is None:
        _NC_CACHE = build()
    return _NC_CACHE


def _bf16(x):
    x32 = np.ascontiguousarray(x, np.float32)
    u = x32.view(np.uint32)
    return (((u + 0x7FFF + ((u >> 16) & 1)) & 0xFFFF0000).astype(np.uint32)).view(np.float32)


def _split3(a):
    hi = _bf16(a)
    r = (a - hi).astype(np.float32)
    mid = _bf16(r)
    lo = _bf16((r - mid).astype(np.float32))
    return hi, mid, lo


def make_in_maps(inputs):
    bf = mybir.dt.np(BF16)
    lt = np.ascontiguousarray(inputs['low_theta'], np.float32)
    lp = np.ascontiguousarray(inputs['low_phi'], np.float32)
    lf = np.ascontiguousarray(inputs['low_feats'], np.float32)
    ht = np.ascontiguousarray(inputs['high_theta'], np.float32)
    hp = np.ascontiguousarray(inputs['high_phi'], np.float32)
    hf = np.ascontiguousarray(inputs['high_feats'], np.float32)
    W0 = np.asarray(inputs['W0'], np.float32)
    W1 = np.asarray(inputs['W1'], np.float32)
    w0lot = np.ascontiguousarray(W0[:, :C1].T).astype(np.float16)   # [64, 128]
    w0hit = np.ascontiguousarray(W0[:, C1:].T).astype(np.float16)   # [128, 128]
    w1t = np.ascontiguousarray(W1.T).astype(np.float16)             # [128, 128]
    g0 = np.ascontiguousarray(np.asarray(inputs['g0'], np.float32).reshape(128, 1))
    be0 = np.ascontiguousarray(np.asarray(inputs['beta0'], np.float32).reshape(128, 1))
    g1 = np.ascontiguousarray(np.asarray(inputs['g1'], np.float32).reshape(128, 1))
    be1 = np.ascontiguousarray(np.asarray(inputs['beta1'], np.float32).reshape(128, 1))

    in_maps = []
    ones = np.ones((1, N), np.float32)
    for b in range(B):
        c0 = np.float32(2.0) * ht[b]
        c1 = np.float32(2.0) * hp[b]
        c2 = -(ht[b] * ht[b] + hp[b] * hp[b])
        a_h, a_m, a_l = _split3(lt[b])
        b_h, b_m, b_l = _split3(c0)
        p_h, p_m, p_l = _split3(lp[b])
        q_h, q_m, q_l = _split3(c1)
        c_h, c_m, c_l = _split3(c2)
        one = ones[0]
        # term rows: products of (query row) x (cand row), accumulated on PE
        qrows = [a_h, a_h, a_m, a_h, a_l, a_m,
                 p_h, p_h, p_m, p_h, p_l, p_m,
                 one, one, one]
        crows = [b_h, b_m, b_h, b_l, b_h, b_m,
                 q_h, q_m, q_h, q_l, q_h, q_m,
                 c_h, c_m, c_l]
        qf15 = np.stack(qrows, axis=0).astype(bf)
        cf15 = np.stack(crows, axis=0).astype(bf)
        in_maps.append({
            "qf15": qf15, "cf15": cf15,
            "lf16": lf[b].astype(np.float16), "hf": hf[b],
            "w0lot": w0lot, "w0hit": w0hit, "w1t": w1t,
            "g0": g0, "be0": be0, "g1": g1, "be1": be1,
        })
    return in_maps


def kernel(**inputs):
    nc = _get_nc()
    in_maps = make_in_maps(inputs)
    res = bass_utils.run_bass_kernel_spmd(nc, in_maps, core_ids=list(range(B)))
    out = np.stack([res.results[b]["out"] for b in range(B)], axis=0)
    return out.astype(np.float32)


# revision 21
# speedup vs baseline: 1.0077x; 1.0077x over previous
"""Trainium2 Bass kernel for AngularFeaturePropagation (retrieval_knn).

Computation per batch element b (one NeuronCore per b, B=8 = n_cores):
  idx[n]  = argmin_m sqrt((lt[n]-ht[m])^2 + (lp[n]-hp[m])^2)      n<8192, m<2048
  interp  = high_feats[:, idx]                                     [128, 8192]
  cat     = [low_feats; interp]                                    [192, 8192]
  y0 = W0 @ cat  -> BN(global batch stats, over all cores) -> ReLU
  y1 = W1 @ h0   -> BN -> ReLU -> out                              [128, 8192]

Device algorithm (v3):
  - scores s[n,m] = 2*lt*ht + 2*lp*hp - (ht^2+hp^2)  (= -dist^2 + const(n))
    via a K=15 bf16 PE matmul (1 cycle/row): every fp32 factor is split
    exactly into hi+mid+lo bf16 terms on the HOST (24 mantissa bits, zero
    split residual), so each bf16 product is exact and only the fp32 PSUM
    accumulation rounds -> near-fp32 scores at bf16 matmul speed.
  - argmax via prefix-max scan + sign-count:
      pm = chained tensor_tensor_scan(max) over the 2048 candidates
      (DVE, the only PSUM consumer -> PSUM freed early);
      idx = #{t : pm[t] < pm[last]} = sum of Sign(pm[last] - pm[t])
      accumulated by the Activation engine's accum_out. Exact fp32
      argmax-first semantics; no rescore stage (argmax of the
      expanded-form scores flips ~2/65536 queries vs the reference).
  - feature gather via GPSIMD ap_gather (channel-major fp32), per
    2-batch chunk, overlapped with the remaining score tiles.
  - MLP in fp16 (weights + low_feats pre-cast on host; interp chunks
    cast by ScalarE; h0 written fp16 by the fused BN0 apply). Layer-0
    BN batch sums (sum y, sum y^2) ride on ScalarE accum_out during the
    PSUM->SBUF copy and a Square pass; layer-1 uses DVE bn_stats in the
    tail. BN: AllReduce of per-device sums; biases dropped (BN
    shift-invariant); affine+ReLU fused into one ScalarE activation.
"""

import sys

if '/opt/trn_rl_repo' not in sys.path:
    sys.path.insert(0, '/opt/trn_rl_repo')

import numpy as np

import concourse.bass as bass
import concourse.bacc as bacc
import concourse.tile as tile
import concourse.mybir as mybir
from concourse import bass_utils, library_config

F32 = mybir.dt.float32
BF16 = mybir.dt.bfloat16
F16 = mybir.dt.float16
U16 = mybir.dt.uint16
I16 = mybir.dt.int16
AF = mybir.ActivationFunctionType
OP = mybir.AluOpType
AX = mybir.AxisListType

B, N, M, C1, C2 = 8, 8192, 2048, 64, 128
NT = N // 128          # 64 query tiles
NCH = N // 512         # 16 MLP chunks
K15 = 15               # bf16 split rows
EPS = 1e-5


def build(num_devices=8):
    nc = bacc.Bacc("TRN2", target_bir_lowering=False, debug=False, num_devices=num_devices)

    # ---- per-core DRAM I/O ----
    d_qf = nc.dram_tensor("qf15", [K15, N], BF16, kind="ExternalInput")
    d_cf = nc.dram_tensor("cf15", [K15, M], BF16, kind="ExternalInput")
    d_lf = nc.dram_tensor("lf16", [C1, N], F16, kind="ExternalInput")
    d_hf = nc.dram_tensor("hf", [C2, M], F32, kind="ExternalInput")
    d_w0lot = nc.dram_tensor("w0lot", [C1, 128], F16, kind="ExternalInput")
    d_w0hit = nc.dram_tensor("w0hit", [C2, 128], F16, kind="ExternalInput")
    d_w1t = nc.dram_tensor("w1t", [128, 128], F16, kind="ExternalInput")
    d_g0 = nc.dram_tensor("g0", [128, 1], F32, kind="ExternalInput")
    d_be0 = nc.dram_tensor("be0", [128, 1], F32, kind="ExternalInput")
    d_g1 = nc.dram_tensor("g1", [128, 1], F32, kind="ExternalInput")
    d_be1 = nc.dram_tensor("be1", [128, 1], F32, kind="ExternalInput")
    d_out = nc.dram_tensor("out", [128, N], F32, kind="ExternalOutput")

    with tile.TileContext(nc) as tc:
        with (
            tc.tile_pool(name="persist", bufs=1) as persist,
            tc.tile_pool(name="dram", bufs=1, space="DRAM") as dram,
            tc.tile_pool(name="small", bufs=2) as small,
            tc.tile_pool(name="lfq", bufs=2) as lfp,
            tc.tile_pool(name="pm", bufs=3) as pmp,
            tc.tile_pool(name="junk", bufs=3) as junkp,
            tc.tile_pool(name="ojunk", bufs=2) as ojunkp,
            tc.tile_pool(name="i16", bufs=2) as i16p,
            tc.tile_pool(name="osb", bufs=2) as osbp,
        ):
            # ---------------- stage 0: loads ----------------
            nc.gpsimd.load_library(library_config.ap_gather)
            qfeat = persist.tile([K15, N], BF16)
            nc.sync.dma_start(qfeat[:], d_qf.ap())
            candfeat = persist.tile([K15, M], BF16)
            nc.sync.dma_start(candfeat[:], d_cf.ap())
            hf_sb = persist.tile([C2, M], F32)
            nc.sync.dma_start(hf_sb[:], d_hf.ap())
            w0lot = persist.tile([C1, 128], F16)
            nc.sync.dma_start(w0lot[:], d_w0lot.ap())
            w0hit = persist.tile([C2, 128], F16)
            nc.sync.dma_start(w0hit[:], d_w0hit.ap())
            w1t = persist.tile([128, 128], F16)
            nc.sync.dma_start(w1t[:], d_w1t.ap())
            g0 = persist.tile([128, 1], F32)
            nc.sync.dma_start(g0[:], d_g0.ap())
            be0 = persist.tile([128, 1], F32)
            nc.sync.dma_start(be0[:], d_be0.ap())
            g1 = persist.tile([128, 1], F32)
            nc.sync.dma_start(g1[:], d_g1.ap())
            be1 = persist.tile([128, 1], F32)
            nc.sync.dma_start(be1[:], d_be1.ap())

            # scan data1 operand (ignored under op1=bypass, but must exist)
            zeros16 = persist.tile([128, 1024], F16)
            nc.vector.memset(zeros16[:], 0.0)

            jall = persist.tile([128, NT], F32)
            idx_u = persist.tile([128, NT], U16)
            d_fi = dram.tile([N], U16)
            fiw = persist.tile([128, N // 16], U16)

            sy0 = persist.tile([128, NCH], F32)
            sy0sq = persist.tile([128, NCH], F32)
            st1 = persist.tile([128, NCH * 6], F32)

            interp = persist.tile([C2, N], F32)
            interp3 = interp[:].rearrange("p (m d) -> p m d", d=1)
            hf3 = hf_sb[:].rearrange("p (m d) -> p m d", d=1)
            y0 = persist.tile([128, N], F16)
            h0 = persist.tile([128, N], F16)
            y1 = persist.tile([128, N], F16)

            # ---------------- phase 1: scores + argmax + L0 ----------------
            with tc.tile_pool(name="mpsum", bufs=2, space="PSUM") as mpsum:
                def l0_chunk(c):
                    # layer-0 matmuls + copies + BN sums for 512-token chunk c
                    q = c // 4
                    if c % 4 == 0:
                        lfq = lfp.tile([C1, 2048], F16, tag="lfq")
                        nc.sync.dma_start(lfq[:], d_lf.ap()[:, 2048 * q:2048 * (q + 1)])
                        l0_chunk.lfq = lfq
                    lfch = l0_chunk.lfq[:, 512 * (c % 4):512 * (c % 4 + 1)]
                    i16 = i16p.tile([128, 512], F16, tag="i16")
                    nc.gpsimd.tensor_copy(i16[:], interp[:, 512 * c:512 * (c + 1)])
                    mps = mpsum.tile([128, 512], F32)
                    nc.tensor.matmul(mps[:], w0lot[:], lfch, start=True, stop=False)
                    nc.tensor.matmul(mps[:], w0hit[:], i16[:], start=False, stop=True)
                    nc.scalar.activation(y0[:, 512 * c:512 * (c + 1)], mps[:], AF.Copy,
                                         accum_out=sy0[:, c:c + 1])
                    oj = ojunkp.tile([128, 512], F16, tag="oj")
                    nc.scalar.activation(oj[:], mps[:], AF.Square,
                                         accum_out=sy0sq[:, c:c + 1])

                spsum_ctx = tc.tile_pool(name="spsum", bufs=3, space="PSUM")
                spsum = spsum_ctx.__enter__()
                for t in range(NT):
                    qT = qfeat[:, 128 * t:128 * (t + 1)]
                    psA = spsum.tile([128, 1024], F32, tag="ps")
                    nc.tensor.matmul(psA[:, 0:512], qT, candfeat[:, 0:512],
                                     start=True, stop=True)
                    nc.tensor.matmul(psA[:, 512:1024], qT, candfeat[:, 512:1024],
                                     start=True, stop=True)
                    psB = spsum.tile([128, 1024], F32, tag="ps")
                    nc.tensor.matmul(psB[:, 0:512], qT, candfeat[:, 1024:1536],
                                     start=True, stop=True)
                    nc.tensor.matmul(psB[:, 512:1024], qT, candfeat[:, 1536:2048],
                                     start=True, stop=True)
                    pm = pmp.tile([128, M], F32, tag="pm")
                    nc.vector.tensor_tensor_scan(pm[:, 0:1024], psA[:], zeros16[:],
                                                 initial=-3.0e38, op0=OP.max, op1=OP.bypass)
                    nc.vector.tensor_tensor_scan(pm[:, 1024:2048], psB[:], zeros16[:],
                                                 initial=pm[:, 1023:1024],
                                                 op0=OP.max, op1=OP.bypass)
                    junk = junkp.tile([128, M], F16, tag="junk")
                    # ScalarE extraction: idx = sum of Sign(r1 - pm)
                    nc.scalar.activation(junk[:], pm[:], AF.Sign,
                                         bias=pm[:, 2047:2048], scale=-1.0,
                                         accum_out=jall[:, t:t + 1])

                    if t % 8 == 3 and t // 8 >= 1:
                        # first deferred layer-0 chunk of the previous batch
                        l0_chunk(2 * (t // 8 - 1))
                    if t % 8 == 7:
                        # batch j of 8 tiles: indices final -> stage + gather;
                        # the previous batch's remaining layer-0 chunk runs now
                        # (one-batch deferral keeps Act's in-order queue from
                        # head-blocking on gather latency).
                        j = t // 8
                        nc.vector.tensor_copy(idx_u[:, 8 * j:8 * (j + 1)],
                                              jall[:, 8 * j:8 * (j + 1)])
                        nc.sync.dma_start(
                            d_fi[1024 * j:1024 * (j + 1)].rearrange("(t p) -> p t", p=128),
                            idx_u[:, 8 * j:8 * (j + 1)])
                        for g in range(8):
                            nc.sync.dma_start(
                                fiw[16 * g:16 * (g + 1), 64 * j:64 * (j + 1)],
                                d_fi[1024 * j:1024 * (j + 1)].rearrange("(s p) -> p s", p=16))
                        nc.gpsimd.ap_gather(
                            interp3[:, 1024 * j:1024 * (j + 1), :], hf3,
                            fiw[:, 64 * j:64 * (j + 1)].bitcast(I16),
                            channels=128, num_elems=M, d=1, num_idxs=1024,
                        )
                        if j >= 1:
                            l0_chunk(2 * (j - 1) + 1)

                # last batch's layer-0 chunks
                l0_chunk(NCH - 2)
                l0_chunk(NCH - 1)
                spsum_ctx.__exit__(None, None, None)

                # ---------------- tail: BN0 -> L1 -> BN1 -> out ----------------
                d_ccin = dram.tile([128, 2], F32)
                d_ccout = dram.tile([128, 2], F32)
                d_ccin1 = dram.tile([128, 2], F32)
                d_ccout1 = dram.tile([128, 2], F32)

                def bn_coeffs(cc_sb, d_in, d_out, gam, bet, tag):
                    # cc_sb [128,2] = per-device (sum, sumsq); AllReduce-add
                    # then mean/E2 -> scale sc, shift sh for fused affine+ReLU.
                    nc.sync.dma_start(d_in[:], cc_sb[:])
                    if tag == "b0":
                        # pull the Sqrt act-table load under the collective wait
                        sqj = small.tile([128, 1], F32, tag="sqj")
                        nc.vector.memset(sqj[:], 1.0)
                        nc.scalar.activation(sqj[:], sqj[:], AF.Sqrt)
                    if num_devices > 1:
                        nc.gpsimd.collective_compute(
                            "AllReduce", OP.add,
                            replica_groups=[list(range(num_devices))],
                            ins=[d_in[:].opt()], outs=[d_out[:].opt()],
                        )
                    else:
                        nc.sync.dma_start(d_out[:], d_in[:])
                    ccr = small.tile([128, 2], F32, tag=tag + "ccr")
                    nc.sync.dma_start(ccr[:], d_out[:])
                    mu = small.tile([128, 1], F32, tag=tag + "mu")
                    nc.vector.tensor_scalar_mul(mu[:], ccr[:, 0:1], 1.0 / (num_devices * N))
                    e2g = small.tile([128, 1], F32, tag=tag + "e2g")
                    nc.vector.tensor_scalar_mul(e2g[:], ccr[:, 1:2], 1.0 / (num_devices * N))
                    musq = small.tile([128, 1], F32, tag=tag + "musq")
                    nc.vector.tensor_mul(musq[:], mu[:], mu[:])
                    var = small.tile([128, 1], F32, tag=tag + "var")
                    nc.vector.tensor_sub(var[:], e2g[:], musq[:])
                    vpe = small.tile([128, 1], F32, tag=tag + "vpe")
                    nc.vector.tensor_scalar_add(vpe[:], var[:], EPS)
                    sd = small.tile([128, 1], F32, tag=tag + "sd")
                    nc.scalar.activation(sd[:], vpe[:], AF.Sqrt)
                    rs = small.tile([128, 1], F32, tag=tag + "rs")
                    nc.vector.reciprocal(rs[:], sd[:])
                    sc = small.tile([128, 1], F32, tag=tag + "sc")
                    nc.vector.tensor_mul(sc[:], gam[:], rs[:])
                    msc = small.tile([128, 1], F32, tag=tag + "msc")
                    nc.vector.tensor_mul(msc[:], mu[:], sc[:])
                    sh = small.tile([128, 1], F32, tag=tag + "sh")
                    nc.vector.tensor_sub(sh[:], bet[:], msc[:])
                    return sc, sh

                # layer-0 stats: sums accumulated on Act during phase 1
                cc0 = small.tile([128, 2], F32, tag="cc0")
                nc.vector.tensor_reduce(cc0[:, 0:1], sy0[:], axis=AX.X, op=OP.add)
                nc.vector.tensor_reduce(cc0[:, 1:2], sy0sq[:], axis=AX.X, op=OP.add)
                sc0, sh0 = bn_coeffs(cc0, d_ccin, d_ccout, g0, be0, "b0")

                # BN0 apply (chunked, h0 in fp16); L1 matmuls + y1 + bn_stats follow
                with tc.tile_pool(name="mp2", bufs=6, space="PSUM") as mp2:
                    for qq in range(4):
                        s2 = slice(2048 * qq, 2048 * (qq + 1))
                        nc.scalar.activation(h0[:, s2], y0[:, s2], AF.Relu,
                                             bias=sh0[:], scale=sc0[:])
                        for c in range(4 * qq, 4 * qq + 4):
                            mps = mp2.tile([128, 512], F32, tag="mp2")
                            nc.tensor.matmul(mps[:], w1t[:], h0[:, 512 * c:512 * (c + 1)],
                                             start=True, stop=True)
                            nc.vector.bn_stats(st1[:, 6 * c:6 * (c + 1)], mps[:])
                            nc.scalar.activation(y1[:, 512 * c:512 * (c + 1)], mps[:], AF.Copy)

                # layer-1 stats from bn_stats/bn_aggr -> (sum, sumsq)
                ag1 = small.tile([128, 2], F32, tag="ag1")
                nc.vector.bn_aggr(ag1[:], st1[:])
                msq1 = small.tile([128, 1], F32, tag="msq1")
                nc.vector.tensor_mul(msq1[:], ag1[:, 0:1], ag1[:, 0:1])
                cc1 = small.tile([128, 2], F32, tag="cc1")
                nc.vector.tensor_scalar_mul(cc1[:, 0:1], ag1[:, 0:1], float(N))
                e2s = small.tile([128, 1], F32, tag="e2s")
                nc.vector.tensor_add(e2s[:], ag1[:, 1:2], msq1[:])
                nc.vector.tensor_scalar_mul(cc1[:, 1:2], e2s[:], float(N))
                sc1, sh1 = bn_coeffs(cc1, d_ccin1, d_ccout1, g1, be1, "b1")

                # BN1 apply + store (1024-wide so the serial store DMA starts early)
                for qq in range(8):
                    s2 = slice(1024 * qq, 1024 * (qq + 1))
                    o_sb = osbp.tile([128, 1024], F32, tag="osb")
                    nc.scalar.activation(o_sb[:], y1[:, s2], AF.Relu,
                                         bias=sh1[:], scale=sc1[:])
                    nc.sync.dma_start(d_out.ap()[:, s2], o_sb[:])

    nc.compile()
    return nc


_NC_CACHE = None


def _get_nc():
    global _NC_CACHE
    if _NC_CACHE is None:
        _NC_CACHE = build()
    return _NC_CACHE


def _bf16(x):
    x32 = np.ascontiguousarray(x, np.float32)
    u = x32.view(np.uint32)
    return (((u + 0x7FFF + ((u >> 16) & 1)) & 0xFFFF0000).astype(np.uint32)).view(np.float32)


def _split3(a):
    hi = _bf16(a)
    r = (a - hi).astype(np.float32)
    mid = _bf16(r)
    lo = _bf16((r - mid).astype(np.float32))
    return hi, mid, lo


def make_in_maps(inputs):
    bf = mybir.dt.np(BF16)
    lt = np.ascontiguousarray(inputs['low_theta'], np.float32)
    lp = np.ascontiguousarray(inputs['low_phi'], np.float32)
    lf = np.ascontiguousarray(inputs['low_feats'], np.float32)
    ht = np.ascontiguousarray(inputs['high_theta'], np.float32)
    hp = np.ascontiguousarray(inputs['high_phi'], np.float32)
    hf = np.ascontiguousarray(inputs['high_feats'], np.float32)
    W0 = np.asarray(inputs['W0'], np.float32)
    W1 = np.asarray(inputs['W1'], np.float32)
    w0lot = np.ascontiguousarray(W0[:, :C1].T).astype(np.float16)   # [64, 128]
    w0hit = np.ascontiguousarray(W0[:, C1:].T).astype(np.float16)   # [128, 128]
    w1t = np.ascontiguousarray(W1.T).astype(np.float16)             # [128, 128]
    g0 = np.ascontiguousarray(np.asarray(inputs['g0'], np.float32).reshape(128, 1))
    be0 = np.ascontiguousarray(np.asarray(inputs['beta0'], np.float32).reshape(128, 1))
    g1 = np.ascontiguousarray(np.asarray(inputs['g1'], np.float32).reshape(128, 1))
    be1 = np.ascontiguousarray(np.asarray(inputs['beta1'], np.float32).reshape(128, 1))

    in_maps = []
    ones = np.ones((1, N), np.float32)
    for b in range(B):
        c0 = np.float32(2.0) * ht[b]
        c1 = np.float32(2.0) * hp[b]
        c2 = -(ht[b] * ht[b] + hp[b] * hp[b])
        a_h, a_m, a_l = _split3(lt[b])
        b_h, b_m, b_l = _split3(c0)
        p_h, p_m, p_l = _split3(lp[b])
        q_h, q_m, q_l = _split3(c1)
        c_h, c_m, c_l = _split3(c2)
        one = ones[0]
        # term rows: products of (query row) x (cand row), accumulated on PE
        qrows = [a_h, a_h, a_m, a_h, a_l, a_m,
                 p_h, p_h, p_m, p_h, p_l, p_m,
                 one, one, one]
        crows = [b_h, b_m, b_h, b_l, b_h, b_m,
                 q_h, q_m, q_h, q_l, q_h, q_m,
                 c_h, c_m, c_l]
        qf15 = np.stack(qrows, axis=0).astype(bf)
        cf15 = np.stack(crows, axis=0).astype(bf)
        in_maps.append({
            "qf15": qf15, "cf15": cf15,
            "lf16": lf[b].astype(np.float16), "hf": hf[b],
            "w0lot": w0lot, "w0hit": w0hit, "w1t": w1t,
            "g0": g0, "be0": be0, "g1": g1, "be1": be1,
        })
    return in_maps


def kernel(**inputs):
    nc = _get_nc()
    in_maps = make_in_maps(inputs)
    res = bass_utils.run_bass_kernel_spmd(nc, in_maps, core_ids=list(range(B)))
    out = np.stack([res.results[b]["out"] for b in range(B)], axis=0)
    return out.astype(np.float32)
